# revision 6
# baseline (speedup 1.0000x reference)
"""Trainium2 Bass kernel for the grouped TF->gene sparse decoder (AEDecoder).

Math (reference):
  h1 = leaky_relu(features[:,:,None] * w1 + b1)            # [B,T,K]
  h2 = leaky_relu(einsum('btj,tjk->btk', h1, w2) + b2)     # [B,T,K]
  out = einsum('bgek,gek->bg', h2[:, edge_tf, :], w3) + b3 # [B,G]

Sparse run-length formulation:
  The final contraction touches only 12 of the 2048 (t,k) rows per gene
  (3 edges x K).  Rows fall in 16 chunks of 128 partitions; a gene touches
  <=3 distinct chunks (avg 2.82).  Genes are sorted globally by their
  (c1<=c2<=c3) chunk triple and dealt round-robin to the 8 cores, so all
  cores share ONE instruction template (run boundaries agree within +-1
  column across cores; padded to the max with zero S-columns) while the S
  data differs per core.  The host un-permutes the gene order at gather.

  Per batch-tile (128 cells), chunks run in ascending order: a gene's
  first chunk writes psum with start=True (level-1 runs contiguous by
  construction; one ambiguous boundary column per block pair gets a
  start@c1 + accum@c1' 1-col pair), later chunks accumulate (level-2/3
  runs contiguous within parent blocks).  Streamed cols ~7.6k/btile vs
  41k dense.  LDWEIGHTS elision (walrus --enable-ldw-opt) keeps the ~45
  same-stationary matmuls per chunk cheap.

  h1 on DVE (tensor_scalar affine + scalar_tensor_tensor leaky); h2 = ACT
  Prelu over the PE block-diag w2 matmul (psum ping-pong banks 5,6).  b3
  is added by a contraction-1 matmul (ones x b3row) closing each psum
  bank; evictions (psum -> bf16 SBUF) alternate ACT/DVE; per-bank out
  DMA.  The 8 psum banks rotate through 4 btiles x 5 bank-slots; btile1's
  bank-7 slot runs early (interleaved into btile0) to hide h2-build
  pacing.  Spack/w2blk stream on the gpsimd DMA queue, featT/cols/b3 on
  the sync queue.

Sharding: 8 cores x 2500 genes (dealt), full batch per core; out bf16
[512, 2500] per core, host casts to fp32 and un-permutes.
"""

import os

import numpy as np
import ml_dtypes

import concourse.bass as bass
import concourse.mybir as mybir
from concourse.bass_utils import run_bass_kernel_spmd

BF16 = mybir.dt.bfloat16
F32 = mybir.dt.float32
AFT = mybir.ActivationFunctionType
ALU = mybir.AluOpType

B, T, K, G, EPG = 512, 512, 4, 20000, 3
NCORES = 8
GSH = G // NCORES            # 2500 genes per core
NCH = (T * K) // 128         # 16 contract chunks
NBT = B // 128               # 4 batch tiles
NSLOT = (GSH + 511) // 512   # 5 psum bank-slots per btile
ALPHA = 0.01

# (btile, slot) -> psum bank ring; b3/eviction order = PE completion order
BANK = lambda m, j: (5 * m + j) % 8
EV_LIST = ([(0, j) for j in range(5)] + [(1, 2), (1, 0), (1, 1), (1, 3), (1, 4)]
           + [(2, j) for j in range(5)] + [(3, j) for j in range(5)])
EV_RANK = {mj: e for e, mj in enumerate(EV_LIST)}

_CACHE = {}
LAST_RESULT = None
_LDW_PATCHED = False


def _enable_ldw_opt():
    """Flip walrus --enable-ldw-opt to true: elides redundant LDWEIGHTS for
    back-to-back matmuls sharing a stationary operand (our per-chunk run
    lists reuse one h2 block across ~45 matmuls)."""
    global _LDW_PATCHED
    if _LDW_PATCHED or os.environ.get("KERNEL_NO_LDWOPT"):
        return
    import concourse.bass_utils as bu
    orig = bu.run_command

    def _run(cmd, **kw):
        new = ["--enable-ldw-opt=true" if c == "--enable-ldw-opt=false" else c
               for c in cmd]
        if new != cmd and os.environ.get("KERNEL_DEBUG"):
            print("[ldw-opt] flag flipped in walrus cmd")
        return orig(new, **kw)

    bu.run_command = _run
    _LDW_PATCHED = True


def _ensure_profile_hook():
    """Register an NTFF profile hook when the image lacks antenv.axon_hooks."""
    import contextlib
    import ctypes
    import sys
    import types

    try:
        import antenv.axon_hooks  # noqa: F401
        return
    except ImportError:
        pass

    holder = {}
    mod = types.ModuleType("antenv.axon_hooks")
    mod.set_axon_ntff_profile_hook = lambda h: holder.__setitem__("h", h)
    mod.get_axon_ntff_profile_hook = lambda: holder.get("h")
    sys.modules["antenv.axon_hooks"] = mod

    so_path = "/opt/axon/libaxon_pjrt.so"
    try:
        lib = ctypes.CDLL(so_path)
    except OSError:
        return
    if not hasattr(lib, "axon_start_nrt_profile"):
        return
    lib.axon_start_nrt_profile.argtypes = [
        ctypes.POINTER(ctypes.c_int64), ctypes.c_size_t,
    ]
    lib.axon_start_nrt_profile.restype = ctypes.c_int64
    lib.axon_stop_nrt_profile.argtypes = [ctypes.c_char_p]
    lib.axon_stop_nrt_profile.restype = ctypes.c_int64

    @contextlib.contextmanager
    def _hook(output_dir, device_ids):
        import jax
        jax.devices()
        if device_ids:
            ids = (ctypes.c_int64 * len(device_ids))(*device_ids)
            rc = lib.axon_start_nrt_profile(ids, len(device_ids))
        else:
            rc = lib.axon_start_nrt_profile(None, 0)
        if rc != 0:
            raise RuntimeError(f"axon_start_nrt_profile rc={rc}")
        try:
            yield
        finally:
            n = lib.axon_stop_nrt_profile(str(output_dir).encode())
            print(f"profile: {n} ntff file(s) written to {output_dir}")

    holder["h"] = _hook

    import concourse.bass_utils as bu
    bu.upload_artifacts = lambda tmpdir: tmpdir


# ---------------------------------------------------------------------------
# Template: global gene sort + round-robin deal -> per-chunk piece lists
# shared by all 8 cores.  Pure function of edge_tf.
# ---------------------------------------------------------------------------

def _build_template(edge_tf):
    chunk = edge_tf // 32                      # [G, EPG]
    keys = np.full((G, 3), 16, np.int64)       # sorted distinct chunks, pad 16
    for g in range(G):
        cs = sorted(set(chunk[g].tolist()))
        keys[g, : len(cs)] = cs
    order = np.lexsort((keys[:, 2], keys[:, 1], keys[:, 0]))
    sk = keys[order]

    def blocks(ncols):
        a = sk[:, :ncols]
        change = np.any(a[1:] != a[:-1], axis=1)
        bounds = [0] + (np.nonzero(change)[0] + 1).tolist() + [len(a)]
        for i in range(len(bounds) - 1):
            yield tuple(a[bounds[i]].tolist()), bounds[i], bounds[i + 1]

    # runs: (chunk, kind, lo, hi, blockkey, level); positions in [0, GSH)
    runs = []
    l1 = list(blocks(1))
    for i, ((c1,), A, Bb) in enumerate(l1):
        lo, hi = (A + 7) // 8, Bb // 8
        if hi > lo:
            runs.append((c1, "start", lo, hi, (c1,), 1))
        if Bb % 8 != 0 and Bb < G:
            c1n = l1[i + 1][0][0]
            runs.append((c1, "amb_s", Bb // 8, Bb // 8 + 1, (c1,), 1))
            runs.append((c1n, "amb_a", Bb // 8, Bb // 8 + 1, (c1n,), 1))
    for (c1, c2), A, Bb in blocks(2):
        if c2 == 16:
            continue
        runs.append((c2, "accum", A // 8, (Bb + 7) // 8, (c1, c2), 2))
    for (c1, c2, c3), A, Bb in blocks(3):
        if c3 == 16:
            continue
        runs.append((c3, "accum", A // 8, (Bb + 7) // 8, (c1, c2, c3), 3))

    # emission order: by chunk ascending; within a chunk starts first
    kindord = {"start": 0, "amb_s": 1, "amb_a": 2, "accum": 3}
    runs.sort(key=lambda r: (r[0], kindord[r[1]], r[2]))

    # spack column offsets (one column set shared by all btiles) and pieces
    # split at psum bank (512-col) boundaries.
    # HW: start=True resets the ENTIRE psum bank, so exactly one piece per
    # bank-slot (the first in emission order) carries start=True; everything
    # else accumulates onto the zeroed bank.
    pieces = []          # (chunk, psum_lo, psum_hi, spack_lo)
    run_off = []         # spack offset of each run, in emission order
    off = 0
    for c, kind, lo, hi, bk, lvl in runs:
        run_off.append(off)
        p = lo
        while p < hi:
            q = min(hi, (p // 512 + 1) * 512)
            pieces.append((c, p, q, off + (p - lo)))
            p = q
        off += hi - lo
    ncols = off

    chunk_pieces = {c: [] for c in range(NCH)}
    slot_seen = set()
    for c, plo, phi, slo in pieces:
        j = plo // 512
        is_start = j not in slot_seen
        slot_seen.add(j)
        chunk_pieces[c].append((is_start, plo, phi, slo))
    # spack DMA piece boundaries: 8 groups of 2 chunks
    grp_hi = []
    for j in range(8):
        hi_c = 2 * j + 1
        nxt = [run_off[i] for i, r in enumerate(runs) if r[0] > hi_c]
        grp_hi.append(min(nxt) if nxt else ncols)

    return dict(keys=keys, order=order, runs=runs, run_off=run_off,
                ncols=ncols, chunk_pieces=chunk_pieces, grp_hi=grp_hi,
                chunkmap=chunk)


# ---------------------------------------------------------------------------
# Host data packing (layout/index preprocessing only)
# ---------------------------------------------------------------------------

def _prep_inputs(tpl, features, w1, b1, w2, b2, w3, b3, edge_tf):
    bf = ml_dtypes.bfloat16
    keys, order, runs = tpl["keys"], tpl["order"], tpl["runs"]
    run_off, ncols = tpl["run_off"], tpl["ncols"]

    featT = np.repeat(np.ascontiguousarray(features.T), K, axis=0)
    featT = np.ascontiguousarray(
        featT.reshape(NCH, 128, B).transpose(1, 0, 2)).astype(bf)

    w1c = w1.reshape(T * K).reshape(NCH, 128).T.astype(np.float32)
    b1c = b1.reshape(T * K).reshape(NCH, 128).T.astype(np.float32)
    b2c = b2.reshape(T * K).reshape(NCH, 128).T.astype(np.float32)
    cols = np.concatenate([w1c, b1c, b2c], axis=1).copy()

    w2r = w2.reshape(NCH, 32, K, K)
    w2blk = np.zeros((NCH, 32, K, 32, K), np.float32)
    for i in range(32):
        w2blk[:, i, :, i, :] = w2r[:, i]
    w2blk = np.ascontiguousarray(
        w2blk.reshape(NCH, 128, 128).transpose(1, 0, 2)).astype(bf)

    # per-gene merged 128-row column per distinct chunk slot
    gcol = np.zeros((G, 3, 128), np.float32)
    gidx = np.arange(G)
    for e in range(EPG):
        t = edge_tf[:, e]
        cc = t // 32
        s = np.argmax(keys == cc[:, None], axis=1)
        rows = 4 * (t % 32)
        for k in range(K):
            np.add.at(gcol, (gidx, s, rows + k), w3[:, e, k])

    gcore = np.empty((NCORES, GSH), np.int64)      # position -> original gene
    for core in range(NCORES):
        gcore[core] = order[np.arange(GSH) * 8 + core]

    spack = np.zeros((NCORES, 128, ncols), np.float32)
    for ri, (c, kind, lo, hi, bk, lvl) in enumerate(runs):
        w = hi - lo
        o = run_off[ri]
        ps = np.arange(lo, hi)
        for core in range(NCORES):
            genes = gcore[core][ps]
            kk = keys[genes]
            member = kk[:, 0] == bk[0]
            for d in range(1, lvl):
                member &= kk[:, d] == bk[d]
            s = np.argmax(kk == c, axis=1)
            vals = np.where(member[:, None], gcol[genes, s, :], 0.0)
            spack[core, :, o : o + w] = vals.T
    spack = spack.astype(bf)

    b3p = np.zeros((NCORES, 1, GSH), np.float32)
    for core in range(NCORES):
        b3p[core, 0, :] = b3[gcore[core]]
    b3p = b3p.astype(bf)

    in_maps = []
    for core in range(NCORES):
        in_maps.append({
            "featT": featT,
            "cols": cols,
            "W2blk": w2blk,
            "Spack": np.ascontiguousarray(spack[core]),
            "B3p": np.ascontiguousarray(b3p[core]),
        })
    return in_maps, gcore


# ---------------------------------------------------------------------------
# Graph
# ---------------------------------------------------------------------------

def _build_graph(tpl):
    from contextlib import ExitStack

    ncols = tpl["ncols"]
    chunk_pieces = tpl["chunk_pieces"]
    grp_hi = tpl["grp_hi"]

    nc = bass.Bass()
    featT_h = nc.declare_dram_parameter("featT", [128, NCH, B], BF16, isOutput=False)
    cols_h = nc.declare_dram_parameter("cols", [128, 3 * NCH], F32, isOutput=False)
    w2blk_h = nc.declare_dram_parameter("W2blk", [128, NCH, 128], BF16, isOutput=False)
    spack_h = nc.declare_dram_parameter("Spack", [128, ncols], BF16, isOutput=False)
    b3p_h = nc.declare_dram_parameter("B3p", [1, GSH], BF16, isOutput=False)
    out_h = nc.declare_dram_parameter("out", [B, GSH], BF16, isOutput=True)

    def slot_w(j):
        return min(GSH - 512 * j, 512)

    with ExitStack() as es:
        featT = es.enter_context(nc.sbuf_tensor("ft_sb", [128, NCH, B], BF16))
        colsb = es.enter_context(nc.sbuf_tensor("cols_sb", [128, 3 * NCH], F32))
        w2blk = es.enter_context(nc.sbuf_tensor("w2_sb", [128, NCH, 128], BF16))
        spk = es.enter_context(nc.sbuf_tensor("spk_sb", [128, ncols], BF16))
        b3sb = es.enter_context(nc.sbuf_tensor("b3_sb", [1, GSH], BF16))
        ones = es.enter_context(nc.sbuf_tensor("ones_sb", [1, 128], BF16))
        tbuf = es.enter_context(nc.sbuf_tensor("t_sb", [128, B], BF16))
        h1 = es.enter_context(nc.sbuf_tensor("h1_sb", [128, NCH, B], BF16))
        h2 = es.enter_context(nc.sbuf_tensor("h2_sb", [128, NCH, B], BF16))
        outsb = es.enter_context(nc.sbuf_tensor("out_sb", [128, NBT, 512 * NSLOT], BF16))
        pm = [es.enter_context(nc.psum_tensor(f"pm{j}", [128, 512], F32))
              for j in range(8)]

        w1a = colsb[:, 0:NCH]
        b1a = colsb[:, NCH : 2 * NCH]
        b2a = colsb[:, 2 * NCH : 3 * NCH]

        with (
            nc.Block() as block,
            nc.semaphore("dsync") as dsync,    # cols/featT/b3 DMA chain
            nc.semaphore("dpool") as dpool,    # w2blk/spack DMA chain
            nc.semaphore("h1s") as sem_h1,     # DVE h1 per chunk
            nc.semaphore("peh") as sem_peh,    # PE w2-mm per chunk
            nc.semaphore("act") as sem_act,    # ACT h2 per chunk
            nc.semaphore("pem") as sem_pem,    # PE bank complete (b3-mm)
            nc.semaphore("evA") as sem_evA,    # ACT evictions
            nc.semaphore("evD") as sem_evD,    # DVE evictions
            nc.semaphore("od") as sem_od,      # out DMA
        ):
            def ev_wait(engine, m, j):
                """Wait for the previous tenant of bank BANK(m,j) to evict."""
                prev = {(1, 3): (0, 0), (1, 4): (0, 1), (2, 0): (0, 2),
                        (2, 1): (0, 3), (2, 2): (0, 4), (2, 3): (1, 0),
                        (2, 4): (1, 1), (3, 0): (1, 2), (3, 1): (1, 3),
                        (3, 2): (1, 4), (3, 3): (2, 0), (3, 4): (2, 1)}.get((m, j))
                if prev is None:
                    return
                e = EV_RANK[prev]
                sem = sem_evA if e % 2 == 0 else sem_evD
                engine.wait_ge(sem, e // 2 + 1)

            @block.sync
            def _(sync: bass.BassEngine):
                sync.dma_start(out=colsb[:], in_=cols_h[:]).then_inc(dsync, 16)
                for q in range(4):
                    sync.dma_start(
                        out=featT[:, 4 * q : 4 * (q + 1), :],
                        in_=featT_h[:, 4 * q : 4 * (q + 1), :],
                    ).then_inc(dsync, 16)
                sync.dma_start(out=b3sb[:], in_=b3p_h[:]).then_inc(dsync, 16)
                for e, (m, j) in enumerate(EV_LIST):
                    sem = sem_evA if e % 2 == 0 else sem_evD
                    sync.wait_ge(sem, e // 2 + 1)
                    w = slot_w(j)
                    sync.dma_start(
                        out=out_h[m * 128 : (m + 1) * 128, 512 * j : 512 * j + w],
                        in_=outsb[:, m, 512 * j : 512 * j + w],
                    ).then_inc(sem_od, 16)
                sync.wait_ge(sem_od, 16 * len(EV_LIST))

            @block.gpsimd
            def _(gp: bass.BassEngine):
                gp.dma_start(out=w2blk[:], in_=w2blk_h[:]).then_inc(dpool, 16)
                lo = 0
                for jj in range(8):
                    hi = grp_hi[jj]
                    if hi > lo:
                        gp.dma_start(
                            out=spk[:, lo:hi], in_=spack_h[:, lo:hi]
                        ).then_inc(dpool, 16)
                    else:
                        gp.dma_start(
                            out=spk[:, lo : lo + 1], in_=spack_h[:, lo : lo + 1]
                        ).then_inc(dpool, 16)
                    lo = hi

            @block.vector
            def _(vector: bass.BassEngine):
                vector.memset(ones[:], 1.0)
                for c in range(NCH):
                    vector.wait_ge(dsync, 32 + 16 * (c // 4))
                    vector.tensor_scalar(
                        tbuf[:], featT[:, c, :], w1a[:, c : c + 1],
                        b1a[:, c : c + 1], ALU.mult, ALU.add,
                    )
                    vector.scalar_tensor_tensor(
                        h1[:, c, :], tbuf[:], ALPHA, tbuf[:], ALU.mult, ALU.max,
                    ).then_inc(sem_h1)
                for e in range(1, len(EV_LIST), 2):
                    m, j = EV_LIST[e]
                    w = slot_w(j)
                    vector.wait_ge(sem_pem, e + 1)
                    vector.tensor_scalar_add(
                        outsb[:, m, 512 * j : 512 * j + w],
                        pm[BANK(m, j)][:, :w], 0.0,
                    ).then_inc(sem_evD)

            @block.scalar
            def _(scalar: bass.BassEngine):
                for c in range(NCH):
                    scalar.wait_ge(sem_peh, c + 1)
                    scalar.activation(
                        h2[:, c, :], pm[5 + c % 2][:, :], AFT.Prelu,
                        bias=b2a[:, c : c + 1], alpha=ALPHA,
                    ).then_inc(sem_act)
                for e in range(0, len(EV_LIST), 2):
                    m, j = EV_LIST[e]
                    w = slot_w(j)
                    scalar.wait_ge(sem_pem, e + 1)
                    scalar.activation(
                        outsb[:, m, 512 * j : 512 * j + w],
                        pm[BANK(m, j)][:, :w], AFT.Copy,
                    ).then_inc(sem_evA)

            @block.tensor
            def _(tensor: bass.BassEngine):
                def warm(k, n=512):
                    for _ in range(k):
                        tensor.matmul(
                            pm[7][:, :n], featT[:, 0, 0:128], featT[:, 0, :n],
                            start=True, stop=True, skip_group_check=True,
                        )

                def emit_runs(m, c, slots):
                    stat = h2[:, c, m * 128 : (m + 1) * 128]
                    for is_start, plo, phi, slo in chunk_pieces[c]:
                        j = plo // 512
                        if j not in slots:
                            continue
                        w = phi - plo
                        tensor.matmul(
                            pm[BANK(m, j)][:, plo - 512 * j : phi - 512 * j],
                            stat, spk[:, slo : slo + w],
                            start=is_start, stop=False, skip_group_check=True,
                        )

                def b3mm(m, j):
                    w = slot_w(j)
                    tensor.matmul(
                        pm[BANK(m, j)][:, :w], ones[0:1, 0:128],
                        b3sb[0:1, 512 * j : 512 * j + w],
                        start=False, stop=True, skip_group_check=True,
                    ).then_inc(sem_pem)

                warm(5)
                # build + btile0 (+ btile1's bank-7 slot j=2)
                for c in range(NCH):
                    if c == 0:
                        tensor.wait_ge(dpool, 16)      # w2blk
                    tensor.wait_ge(sem_h1, c + 1)
                    if c >= 2:
                        tensor.wait_ge(sem_act, c - 1)  # ph bank free
                    tensor.matmul(
                        pm[5 + c % 2][:, :], w2blk[:, c, :], h1[:, c, :],
                        start=True, stop=True,
                    ).then_inc(sem_peh)
                    tensor.wait_ge(sem_act, c + 1)
                    tensor.wait_ge(dpool, 16 * (c // 2 + 2))  # spack group
                    emit_runs(0, c, (0, 1, 2, 3, 4))
                    emit_runs(1, c, (2,))
                tensor.wait_ge(dsync, 96)              # b3sb
                for j in range(5):
                    b3mm(0, j)
                b3mm(1, 2)
                # btile1 slots 0,1 (banks 5,6 -- free once ACT consumed ph)
                for c in range(NCH):
                    emit_runs(1, c, (0, 1))
                b3mm(1, 0)
                b3mm(1, 1)
                # btile1 slots 3,4 (banks 0,1 <- evictions of t0 j0,j1)
                ev_wait(tensor, 1, 3)
                ev_wait(tensor, 1, 4)
                for c in range(NCH):
                    emit_runs(1, c, (3, 4))
                b3mm(1, 3)
                b3mm(1, 4)
                # btile2
                for j in range(5):
                    ev_wait(tensor, 2, j)
                for c in range(NCH):
                    emit_runs(2, c, (0, 1, 2, 3, 4))
                for j in range(5):
                    b3mm(2, j)
                # btile3
                for j in range(5):
                    ev_wait(tensor, 3, j)
                for c in range(NCH):
                    emit_runs(3, c, (0, 1, 2, 3, 4))
                for j in range(5):
                    b3mm(3, j)

    return nc


def kernel(features, w1, b1, w2, b2, w3, b3, edge_tf):
    global LAST_RESULT
    features, w1, b1, w2, b2, w3, b3, edge_tf = (
        np.asarray(x) for x in (features, w1, b1, w2, b2, w3, b3, edge_tf)
    )
    key = hash(edge_tf.tobytes())
    if key not in _CACHE:
        tpl = _build_template(edge_tf)
        _CACHE.clear()
        _CACHE[key] = (tpl, _build_graph(tpl))
    tpl, graph = _CACHE[key]

    in_maps, gcore = _prep_inputs(
        tpl, features, w1, b1, w2, b2, w3, b3, edge_tf)
    trace = bool(int(os.environ.get("KERNEL_TRACE", "0")))
    if trace:
        _ensure_profile_hook()
    _enable_ldw_opt()
    res = run_bass_kernel_spmd(
        graph, in_maps, core_ids=list(range(NCORES)), trace=trace,
    )
    LAST_RESULT = res
    out = np.zeros((B, G), np.float32)
    for core in range(NCORES):
        dev = np.asarray(res.results[core]["out"]).astype(np.float32)
        out[:, gcore[core]] = dev
    return out


# revision 25
# speedup vs baseline: 1.0971x; 1.0971x over previous
"""Trainium2 Bass kernel for the grouped TF->gene sparse decoder (AEDecoder).

Math (reference):
  h1 = leaky_relu(features[:,:,None] * w1 + b1)            # [B,T,K]
  h2 = leaky_relu(einsum('btj,tjk->btk', h1, w2) + b2)     # [B,T,K]
  out = einsum('bgek,gek->bg', h2[:, edge_tf, :], w3) + b3 # [B,G]

Device formulation (per core, raw-Bass Block with explicit semaphores):
  - Everything transposed: contract dim c=(t,k) = 2048 rows = 16 chunks of
    128 partitions; batch on the free axis.
  - h1T_c = Prelu(featT_c * w1_c + b1_c)           (ACT, per-partition scale/bias)
  - h2T_c = Prelu(W2blk_c^T @ h1T_c + b2_c)        (PE block-diag 4x4 + ACT)
  - out[b, g] = sum_c h2T[c, b]*S[c, g] + b3[g]    (PE dense bf16 matmul)
    S is the host-packed scatter of w3 over (t,k) rows: S[4t+k, g] += w3[g,e,k]
    for each edge e with edge_tf[g,e]==t. Sparsity (12 nnz/col) is not
    exploitable on the PE (random TF spread), so the contraction runs dense.
  - 20 output tiles [128b x 512g] pipelined through a 7-bank PSUM ring with
    a bank-aware staggered schedule (tiles 0-4 open immediately on the pm
    banks, 5-6 after the h2-build vacates the ph banks, refills open as
    banks free up); the h2-build itself is interleaved one chunk per
    schedule step so the main contraction starts ~5 us earlier; DVE adds b3
    on eviction (stop-emission order); per-tile out DMA.
  - S (10.5 MB bf16/core) streams in 8 DMAs with per-piece semaphores so the
    PE starts after ~1/8 of S has landed; warmup matmuls keep the PE HAM
    activity window fed (else it drops to the 1.2 GHz p-state during the
    DMA-paced phase).

Sharding: 8 cores = 8 gene-groups (2500 genes each), full batch per core.
S / b3 differ per core; features (k-replicated, transposed) and w1/b1/w2/b2
are replicated. Host does layout/index packing only; all FLOPs on device.
"""

import os

import numpy as np
import ml_dtypes

import concourse.bass as bass
import concourse.mybir as mybir
import concourse.tile as tile
from concourse.bass_utils import run_bass_kernel_spmd

BF16 = mybir.dt.bfloat16
F32 = mybir.dt.float32
AFT = mybir.ActivationFunctionType

B, T, K, G, EPG = 512, 512, 4, 20000, 3
NCORES = 8
BGRP, GGRP = 1, 8            # batch-groups x gene-groups
BSH = B // BGRP              # 512 cells per core (full batch)
GSH = G // GGRP              # 2500 genes per core
NCH = (T * K) // 128         # 16 contract chunks
GT = 512                     # gene tile (matmul free dim)
GP = ((GSH + GT - 1) // GT) * GT   # 10240 padded genes
NGT = GP // GT               # 20 gene tiles

ALPHA = 0.01                 # leaky_relu slope

_GRAPH = None
LAST_RESULT = None
_LDW_PATCHED = False


def _enable_ldw_opt():
    """Flip walrus --enable-ldw-opt to true: elides redundant LDWEIGHTS for
    back-to-back matmuls that share the same stationary operand (our inner
    gene-tile loop reuses one h2 block across NGT matmuls)."""
    global _LDW_PATCHED
    if _LDW_PATCHED or not os.environ.get("KERNEL_LDW_OPT"):
        return
    import concourse.bass_utils as bu
    orig = bu.run_command

    def _run(cmd, **kw):
        cmd = ["--enable-ldw-opt=true" if c == "--enable-ldw-opt=false" else c
               for c in cmd]
        return orig(cmd, **kw)

    bu.run_command = _run
    _LDW_PATCHED = True


def _ensure_profile_hook():
    """Register an NTFF profile hook when the image lacks antenv.axon_hooks.

    Replicates trn_agent_boot's ctypes shim against libaxon_pjrt.so so
    run_bass_kernel_spmd(trace=True) can capture exec_time_ns.
    """
    import contextlib
    import ctypes
    import sys
    import types

    try:
        import antenv.axon_hooks  # noqa: F401
        return
    except ImportError:
        pass

    holder = {}
    mod = types.ModuleType("antenv.axon_hooks")
    mod.set_axon_ntff_profile_hook = lambda h: holder.__setitem__("h", h)
    mod.get_axon_ntff_profile_hook = lambda: holder.get("h")
    sys.modules["antenv.axon_hooks"] = mod

    so_path = "/opt/axon/libaxon_pjrt.so"
    try:
        lib = ctypes.CDLL(so_path)
    except OSError:
        return
    if not hasattr(lib, "axon_start_nrt_profile"):
        return
    lib.axon_start_nrt_profile.argtypes = [
        ctypes.POINTER(ctypes.c_int64), ctypes.c_size_t,
    ]
    lib.axon_start_nrt_profile.restype = ctypes.c_int64
    lib.axon_stop_nrt_profile.argtypes = [ctypes.c_char_p]
    lib.axon_stop_nrt_profile.restype = ctypes.c_int64

    @contextlib.contextmanager
    def _hook(output_dir, device_ids):
        import jax
        jax.devices()
        if device_ids:
            ids = (ctypes.c_int64 * len(device_ids))(*device_ids)
            rc = lib.axon_start_nrt_profile(ids, len(device_ids))
        else:
            rc = lib.axon_start_nrt_profile(None, 0)
        if rc != 0:
            raise RuntimeError(f"axon_start_nrt_profile rc={rc}")
        try:
            yield
        finally:
            n = lib.axon_stop_nrt_profile(str(output_dir).encode())
            print(f"profile: {n} ntff file(s) written to {output_dir}")

    holder["h"] = _hook

    import concourse.bass_utils as bu
    bu.upload_artifacts = lambda tmpdir: tmpdir


def _build_graph():
    from contextlib import ExitStack

    nc = bass.Bass()

    featT_h = nc.declare_dram_parameter("featT", [128, NCH, BSH], BF16, isOutput=False)
    s_h = nc.declare_dram_parameter("S", [128, NCH, GP], BF16, isOutput=False)
    w2blk_h = nc.declare_dram_parameter("W2blk", [128, NCH, 128], BF16, isOutput=False)
    cols_h = nc.declare_dram_parameter("cols", [128, 3 * NCH], F32, isOutput=False)
    b3rep_h = nc.declare_dram_parameter("b3rep", [128, GP], F32, isOutput=False)
    out_h = nc.declare_dram_parameter("out", [B, GSH], F32, isOutput=True)

    NBT = BSH // 128          # 4 b-tiles
    NT = NGT * NBT            # 20 (gene-tile, b-tile) output tiles
    NPM = 5                   # main psum ring (one per gene tile)
    N_CONST_DMAS = 4          # cols, b3rep, featT, w2blk

    with ExitStack() as es:
        s_sb = es.enter_context(nc.sbuf_tensor("s_sb", [128, NCH, GP], BF16))
        featT = es.enter_context(nc.sbuf_tensor("ft_sb", [128, NCH, BSH], BF16))
        h1 = es.enter_context(nc.sbuf_tensor("h1_sb", [128, NCH, BSH], BF16))
        h2 = es.enter_context(nc.sbuf_tensor("h2_sb", [128, NCH, BSH], BF16))
        w2blk = es.enter_context(nc.sbuf_tensor("w2_sb", [128, NCH, 128], BF16))
        cols = es.enter_context(nc.sbuf_tensor("cols_sb", [128, 3 * NCH], F32))
        b3rep = es.enter_context(nc.sbuf_tensor("b3rep_sb", [128, GP], F32))
        ot = [es.enter_context(nc.sbuf_tensor(f"ot{j}", [128, GP], F32)) for j in range(NBT)]
        ph = [es.enter_context(nc.psum_tensor(f"ph{j}", [128, BSH], F32)) for j in range(2)]
        pm = [es.enter_context(nc.psum_tensor(f"pm{j}", [128, GT], F32)) for j in range(NPM)]
        pwarm = es.enter_context(nc.psum_tensor("pwarm", [128, GT], F32))

        w1c = cols[:, 0:NCH]
        b1c = cols[:, NCH : 2 * NCH]
        b2c = cols[:, 2 * NCH : 3 * NCH]

        class _Sched:  # shared schedule holder
            pass
        tc = _Sched()
        NT_ALL0 = (BSH // 128) * NGT
        _start = {0: 1, 1: 2, 2: 3, 3: 4, 4: 5, 5: NCH, 6: NCH + 1}
        _bank = {j: j for j in range(7)}
        _prev = {}
        _ten = list(range(7))
        _free = [_start[j] + NCH for j in range(7)]
        for _j in range(7, NT_ALL0):
            _b = min(range(7), key=lambda x: _free[x])
            _s = max(_free[_b] + 1, _start[_j - 1] + 2)
            _bank[_j] = _b
            _start[_j] = _s
            _prev[_j] = _ten[_b]
            _ten[_b] = _j
            _free[_b] = _s + NCH
        _eorder = sorted(range(NT_ALL0), key=lambda t: _start[t])
        _erank = {t: i for i, t in enumerate(_eorder)}
        tc.SCHED = (_start, _bank, _prev, _eorder, _erank)

        with (
            nc.Block() as block,
            nc.semaphore("consts") as sem_consts,
            nc.semaphore("sg0") as sg0,
            nc.semaphore("sg1") as sg1,
            nc.semaphore("sg2") as sg2,
            nc.semaphore("sg3") as sg3,
            nc.semaphore("act") as sem_act,
            nc.semaphore("peh") as sem_peh,
            nc.semaphore("pem") as sem_pem,
            nc.semaphore("ev") as sem_ev,
            nc.semaphore("od") as sem_od,
            nc.semaphore("b3") as sem_b3,
            nc.semaphore("ft0") as ft0,
            nc.semaphore("ft1") as ft1,
            nc.semaphore("ft2") as ft2,
            nc.semaphore("ft3") as ft3,
            nc.semaphore("sh0") as sh0,
            nc.semaphore("sh1") as sh1,
            nc.semaphore("sh2") as sh2,
            nc.semaphore("sh3") as sh3,
        ):
            sft = [ft0, ft1, ft2, ft3]
            shalf = [sh0, sh1, sh2, sh3]  # second half of each S group
            sgrp = [sg0, sg1, sg2, sg3]

            @block.sync
            def _(sync: bass.BassEngine):
                sync.dma_start(out=cols[:], in_=cols_h[:]).then_inc(sem_consts, 16)
                sync.dma_start(out=w2blk[:], in_=w2blk_h[:]).then_inc(sem_consts, 16)
                sync.dma_start(
                    out=featT[:, 0:4, :], in_=featT_h[:, 0:4, :]
                ).then_inc(ft0, 16)
                sync.dma_start(
                    out=s_sb[:, 0:2, :], in_=s_h[:, 0:2, :]
                ).then_inc(sgrp[0], 16)
                sync.dma_start(
                    out=s_sb[:, 2:4, :], in_=s_h[:, 2:4, :]
                ).then_inc(shalf[0], 16)
                for p in range(1, 4):
                    sync.dma_start(
                        out=featT[:, 4 * p : 4 * (p + 1), :],
                        in_=featT_h[:, 4 * p : 4 * (p + 1), :],
                    ).then_inc(sft[p], 16)
                for j in range(1, 4):
                    sync.dma_start(
                        out=s_sb[:, 4 * j : 4 * j + 2, :],
                        in_=s_h[:, 4 * j : 4 * j + 2, :],
                    ).then_inc(sgrp[j], 16)
                    sync.dma_start(
                        out=s_sb[:, 4 * j + 2 : 4 * j + 4, :],
                        in_=s_h[:, 4 * j + 2 : 4 * j + 4, :],
                    ).then_inc(shalf[j], 16)
                sync.dma_start(out=b3rep[:], in_=b3rep_h[:]).then_inc(sem_b3, 16)
                _, _, _, evict_order, _ = tc.SCHED
                for i, t in enumerate(evict_order):
                    m, n = t // NGT, t % NGT
                    w = min(GSH - n * GT, GT)
                    sync.wait_ge(sem_ev, i + 1)
                    sync.dma_start(
                        out=out_h[m * 128 : (m + 1) * 128, n * GT : n * GT + w],
                        in_=ot[m][:, n * GT : n * GT + w],
                    ).then_inc(sem_od, 16)
                sync.wait_ge(sem_od, 16 * NBT * NGT)

            @block.scalar
            def _(scalar: bass.BassEngine):
                scalar.wait_ge(sem_consts, 32)
                for c in range(NCH):
                    if c % 4 == 0:
                        scalar.wait_ge(sft[c // 4], 16)
                    scalar.activation(
                        h1[:, c, :], featT[:, c, :], AFT.Prelu,
                        bias=b1c[:, c : c + 1], scale=w1c[:, c : c + 1], alpha=ALPHA,
                    ).then_inc(sem_act)
                    scalar.wait_ge(sem_peh, c + 1)
                    scalar.activation(
                        h2[:, c, :], ph[c % 2][:], AFT.Prelu,
                        bias=b2c[:, c : c + 1], alpha=ALPHA,
                    ).then_inc(sem_act)

            @block.tensor
            def _(tensor: bass.BassEngine):
                def warm(k, n=BSH):
                    # keep the PE activity window fed so HAM ramps to full clock
                    for _ in range(k):
                        tensor.matmul(
                            pwarm[:, :n], featT[:, 0, 0:128], featT[:, 0, :n],
                            start=True, stop=True, skip_group_check=True,
                        )
                warm(16)  # spin from t=0 (garbage reads) so HAM ramps early
                pm7 = pm + ph
                NT_ALL = NBT * NGT
                start_step, bank_of, prev_tenant, evict_order, evict_rank = tc.SCHED

                sgrp_waited = [False] * 8
                act_waited = [False] * NCH
                h2_built = 0
                n_steps = max(start_step.values()) + NCH
                for s in range(n_steps):
                    if h2_built < NCH:
                        c = h2_built
                        if c >= 4:
                            warm(1, 128)  # bridge ACT latency, keep HAM fed
                        tensor.wait_ge(sem_act, 2 * c + 1)
                        tensor.matmul(
                            ph[c % 2][:], w2blk[:, c, :], h1[:, c, :],
                            start=True, stop=True,
                        ).then_inc(sem_peh)
                        if s == 0:
                            warm(3)
                        h2_built += 1
                    for t in range(NT_ALL):
                        ci = s - start_step[t]
                        if ci < 0 or ci >= NCH:
                            continue
                        m, n = t // NGT, t % NGT
                        w = min(GSH - n * GT, GT)
                        gsl = slice(n * GT, n * GT + w)
                        c = ci
                        if ci == 0:
                            if t in prev_tenant:
                                tensor.wait_ge(sem_ev, evict_rank[prev_tenant[t]] + 1)
                            if t in (5, 6):
                                # ph bank: wait for the h2-build epilogue on ACT
                                tensor.wait_ge(sem_act, 2 * NCH)
                        if not act_waited[c]:
                            tensor.wait_ge(sem_act, 2 * c + 2)
                            act_waited[c] = True
                        if not sgrp_waited[c // 2]:
                            if c < 4:
                                warm(8)
                            else:
                                warm(2, 128)
                            sem = sgrp[c // 4] if (c % 4) < 2 else shalf[c // 4]
                            tensor.wait_ge(sem, 16)
                            sgrp_waited[c // 2] = True
                        mm = tensor.matmul(
                            pm7[bank_of[t]][:, :w],
                            h2[:, c, m * 128 : (m + 1) * 128],
                            s_sb[:, c, gsl],
                            start=(ci == 0), stop=(ci == NCH - 1),
                            skip_group_check=True,
                        )
                        if ci == NCH - 1:
                            mm.then_inc(sem_pem)

            @block.vector
            def _(vector: bass.BassEngine):
                _, bank_of, _, evict_order, _ = tc.SCHED
                vector.wait_ge(sem_b3, 16)
                for i, t in enumerate(evict_order):
                    m, n = t // NGT, t % NGT
                    w = min(GSH - n * GT, GT)
                    vector.wait_ge(sem_pem, i + 1)
                    vector.tensor_add(
                        ot[m][:, n * GT : n * GT + w], (pm + ph)[bank_of[t]][:, :w],
                        b3rep[:, n * GT : n * GT + w],
                    ).then_inc(sem_ev)

    return nc


def _prep_inputs(features, w1, b1, w2, b2, w3, b3, edge_tf):
    """Host-side packing: layout/index preprocessing only."""
    bf = ml_dtypes.bfloat16
    featT = np.repeat(np.ascontiguousarray(features.T), K, axis=0)  # [2048, B]
    featT = np.ascontiguousarray(
        featT.reshape(NCH, 128, B).transpose(1, 0, 2)).astype(bf)  # [128, NCH, B]

    w1c = w1.reshape(T * K).reshape(NCH, 128).T.astype(np.float32)
    b1c = b1.reshape(T * K).reshape(NCH, 128).T.astype(np.float32)
    b2c = b2.reshape(T * K).reshape(NCH, 128).T.astype(np.float32)
    cols = np.concatenate([w1c, b1c, b2c], axis=1).copy()

    w2r = w2.reshape(NCH, 32, K, K)
    w2blk = np.zeros((NCH, 32, K, 32, K), np.float32)
    for i in range(32):
        w2blk[:, i, :, i, :] = w2r[:, i]
    w2blk = np.ascontiguousarray(
        w2blk.reshape(NCH, 128, 128).transpose(1, 0, 2)).astype(bf)

    s_gg, b3_gg = [], []
    for gg in range(GGRP):
        gsl = slice(gg * GSH, (gg + 1) * GSH)
        et = edge_tf[gsl]                      # [GSH, EPG]
        wv = w3[gsl].astype(np.float32)        # [GSH, EPG, K]
        s = np.zeros((T * K, GP), np.float32)
        rows = (et[:, :, None] * K + np.arange(K)[None, None, :])  # [GSH,EPG,K]
        scols = np.broadcast_to(np.arange(GSH)[:, None, None], rows.shape)
        np.add.at(s, (rows.ravel(), scols.ravel()), wv.ravel())
        s_gg.append(np.ascontiguousarray(
            s.reshape(NCH, 128, GP).transpose(1, 0, 2)).astype(bf))
        b3p = np.zeros((GP,), np.float32)
        b3p[:GSH] = b3[gsl]
        b3_gg.append(np.ascontiguousarray(np.broadcast_to(b3p, (128, GP))))

    in_maps = []
    for core in range(NCORES):
        gg = core
        in_maps.append({
            "featT": featT,
            "S": s_gg[gg],
            "W2blk": w2blk,
            "cols": cols,
            "b3rep": b3_gg[gg],
        })
    return in_maps


def kernel(features, w1, b1, w2, b2, w3, b3, edge_tf):
    global _GRAPH, LAST_RESULT
    features, w1, b1, w2, b2, w3, b3, edge_tf = (
        np.asarray(x) for x in (features, w1, b1, w2, b2, w3, b3, edge_tf)
    )
    if _GRAPH is None:
        _GRAPH = _build_graph()
    in_maps = _prep_inputs(features, w1, b1, w2, b2, w3, b3, edge_tf)
    trace = bool(int(os.environ.get("KERNEL_TRACE", "0")))
    if trace:
        _ensure_profile_hook()
    _enable_ldw_opt()
    res = run_bass_kernel_spmd(
        _GRAPH, in_maps, core_ids=list(range(NCORES)), trace=trace,
    )
    LAST_RESULT = res
    out = np.zeros((B, G), np.float32)
    for core in range(NCORES):
        out[:, core * GSH : (core + 1) * GSH] = (
            np.asarray(res.results[core]["out"]).astype(np.float32)
        )
    return out



# revision 27
# speedup vs baseline: 1.3684x; 1.2473x over previous
"""Trainium2 Bass kernel for the grouped TF->gene sparse decoder (AEDecoder).

Math (reference):
  h1 = leaky_relu(features[:,:,None] * w1 + b1)            # [B,T,K]
  h2 = leaky_relu(einsum('btj,tjk->btk', h1, w2) + b2)     # [B,T,K]
  out = einsum('bgek,gek->bg', h2[:, edge_tf, :], w3) + b3 # [B,G]

Sparse run-length formulation:
  The final contraction touches only 12 of the 2048 (t,k) rows per gene
  (3 edges x K).  Rows fall in 16 chunks of 128 partitions; a gene touches
  <=3 distinct chunks (avg 2.82).  Genes are sorted globally by their
  (c1<=c2<=c3) chunk triple and dealt round-robin to the 8 cores, so all
  cores share ONE instruction template (run boundaries agree within +-1
  column across cores; padded to the max with zero S-columns) while the S
  data differs per core.  The host un-permutes the gene order at gather.

  Per batch-tile (128 cells), chunks run in ascending order: a gene's
  first chunk writes psum with start=True (level-1 runs contiguous by
  construction; one ambiguous boundary column per block pair gets a
  start@c1 + accum@c1' 1-col pair), later chunks accumulate (level-2/3
  runs contiguous within parent blocks).  Streamed cols ~7.6k/btile vs
  41k dense.  LDWEIGHTS elision (walrus --enable-ldw-opt) keeps the ~45
  same-stationary matmuls per chunk cheap.

  h1 on DVE (tensor_scalar affine + scalar_tensor_tensor leaky); h2 = ACT
  Prelu over the PE block-diag w2 matmul (psum ping-pong banks 5,6).  b3
  is added by a contraction-1 matmul (ones x b3row) closing each psum
  bank; evictions (psum -> bf16 SBUF) alternate ACT/DVE; per-bank out
  DMA.  The 8 psum banks rotate through 4 btiles x 5 bank-slots; btile1's
  bank-7 slot runs early (interleaved into btile0) to hide h2-build
  pacing.  Spack/w2blk stream on the gpsimd DMA queue, featT/cols/b3 on
  the sync queue.

Sharding: 8 cores x 2500 genes (dealt), full batch per core; out bf16
[512, 2500] per core, host casts to fp32 and un-permutes.
"""

import os

import numpy as np
import ml_dtypes

import concourse.bass as bass
import concourse.mybir as mybir
from concourse.bass_utils import run_bass_kernel_spmd

BF16 = mybir.dt.bfloat16
F32 = mybir.dt.float32
AFT = mybir.ActivationFunctionType
ALU = mybir.AluOpType

B, T, K, G, EPG = 512, 512, 4, 20000, 3
NCORES = 8
GSH = G // NCORES            # 2500 genes per core
NCH = (T * K) // 128         # 16 contract chunks (h-build granularity)
NSC = 8                      # 8 superchunks of 256 rows for the main matmul
SUBS = 2                     # partition chunks per superchunk
NBT = B // 128               # 4 batch tiles
NSLOT = (GSH + 511) // 512   # 5 psum bank-slots per btile
ALPHA = 0.01

# (btile, slot) -> psum bank ring; b3/eviction order = PE completion order
BANK = lambda m, j: (5 * m + j) % 8
EV_LIST = ([(0, j) for j in range(5)] + [(1, 2), (1, 0), (1, 1), (1, 3), (1, 4)]
           + [(2, j) for j in range(5)] + [(3, j) for j in range(5)])
EV_RANK = {mj: e for e, mj in enumerate(EV_LIST)}

_CACHE = {}
LAST_RESULT = None
_LDW_PATCHED = False


def _enable_ldw_opt():
    """Flip walrus --enable-ldw-opt to true: elides redundant LDWEIGHTS for
    back-to-back matmuls sharing a stationary operand (our per-chunk run
    lists reuse one h2 block across ~45 matmuls)."""
    global _LDW_PATCHED
    if _LDW_PATCHED or not os.environ.get("KERNEL_LDW_OPT"):
        return
    import concourse.bass_utils as bu
    orig = bu.run_command

    def _run(cmd, **kw):
        new = ["--enable-ldw-opt=true" if c == "--enable-ldw-opt=false" else c
               for c in cmd]
        if new != cmd and os.environ.get("KERNEL_DEBUG"):
            print("[ldw-opt] flag flipped in walrus cmd")
        return orig(new, **kw)

    bu.run_command = _run
    _LDW_PATCHED = True


def _ensure_profile_hook():
    """Register an NTFF profile hook when the image lacks antenv.axon_hooks."""
    import contextlib
    import ctypes
    import sys
    import types

    try:
        import antenv.axon_hooks  # noqa: F401
        return
    except ImportError:
        pass

    holder = {}
    mod = types.ModuleType("antenv.axon_hooks")
    mod.set_axon_ntff_profile_hook = lambda h: holder.__setitem__("h", h)
    mod.get_axon_ntff_profile_hook = lambda: holder.get("h")
    sys.modules["antenv.axon_hooks"] = mod

    so_path = "/opt/axon/libaxon_pjrt.so"
    try:
        lib = ctypes.CDLL(so_path)
    except OSError:
        return
    if not hasattr(lib, "axon_start_nrt_profile"):
        return
    lib.axon_start_nrt_profile.argtypes = [
        ctypes.POINTER(ctypes.c_int64), ctypes.c_size_t,
    ]
    lib.axon_start_nrt_profile.restype = ctypes.c_int64
    lib.axon_stop_nrt_profile.argtypes = [ctypes.c_char_p]
    lib.axon_stop_nrt_profile.restype = ctypes.c_int64

    @contextlib.contextmanager
    def _hook(output_dir, device_ids):
        import jax
        jax.devices()
        if device_ids:
            ids = (ctypes.c_int64 * len(device_ids))(*device_ids)
            rc = lib.axon_start_nrt_profile(ids, len(device_ids))
        else:
            rc = lib.axon_start_nrt_profile(None, 0)
        if rc != 0:
            raise RuntimeError(f"axon_start_nrt_profile rc={rc}")
        try:
            yield
        finally:
            n = lib.axon_stop_nrt_profile(str(output_dir).encode())
            print(f"profile: {n} ntff file(s) written to {output_dir}")

    holder["h"] = _hook

    import concourse.bass_utils as bu
    bu.upload_artifacts = lambda tmpdir: tmpdir


# ---------------------------------------------------------------------------
# Template: global gene sort + round-robin deal -> per-chunk piece lists
# shared by all 8 cores.  Pure function of edge_tf.
# ---------------------------------------------------------------------------

def _build_template(edge_tf):
    chunk = edge_tf // 64                      # [G, EPG] superchunk (256 rows)
    keys = np.full((G, 3), NSC, np.int64)      # sorted distinct, pad NSC
    for g in range(G):
        cs = sorted(set(chunk[g].tolist()))
        keys[g, : len(cs)] = cs
    order = np.lexsort((keys[:, 2], keys[:, 1], keys[:, 0]))
    sk = keys[order]

    def blocks(ncols):
        a = sk[:, :ncols]
        change = np.any(a[1:] != a[:-1], axis=1)
        bounds = [0] + (np.nonzero(change)[0] + 1).tolist() + [len(a)]
        for i in range(len(bounds) - 1):
            yield tuple(a[bounds[i]].tolist()), bounds[i], bounds[i + 1]

    # runs: (sc, kind, lo, hi, blockkey, level); positions in [0, GSH)
    runs = []
    l1 = list(blocks(1))
    for i, ((c1,), A, Bb) in enumerate(l1):
        lo, hi = (A + 7) // 8, Bb // 8
        if hi > lo:
            runs.append((c1, "start", lo, hi, (c1,), 1))
        if Bb % 8 != 0 and Bb < G:
            c1n = l1[i + 1][0][0]
            runs.append((c1, "amb_s", Bb // 8, Bb // 8 + 1, (c1,), 1))
            runs.append((c1n, "amb_a", Bb // 8, Bb // 8 + 1, (c1n,), 1))
    for (c1, c2), A, Bb in blocks(2):
        if c2 == NSC:
            continue
        runs.append((c2, "accum", A // 8, (Bb + 7) // 8, (c1, c2), 2))
    for (c1, c2, c3), A, Bb in blocks(3):
        if c3 == NSC:
            continue
        runs.append((c3, "accum", A // 8, (Bb + 7) // 8, (c1, c2, c3), 3))

    # emission order: by superchunk ascending; within one, starts first
    kindord = {"start": 0, "amb_s": 1, "amb_a": 2, "accum": 3}
    runs.sort(key=lambda r: (r[0], kindord[r[1]], r[2]))

    # Each run expands to SUBS matmuls (contraction 256 = 2 partition chunks);
    # spack stores the run's sub-0 block then sub-1 block.  Pieces split at
    # psum bank (512-col) boundaries.
    # HW: start=True resets the ENTIRE psum bank, so exactly one matmul per
    # bank-slot (the first in emission order) carries start=True; everything
    # else accumulates onto the zeroed bank.
    pieces = []          # (sc, psum_lo, psum_hi, spack_lo_run, run_lo, width)
    run_off = []         # spack offset of each run (sub-0 block)
    off = 0
    for c, kind, lo, hi, bk, lvl in runs:
        run_off.append(off)
        p = lo
        while p < hi:
            q = min(hi, (p // 512 + 1) * 512)
            pieces.append((c, p, q, off, lo, hi - lo))
            p = q
        off += SUBS * (hi - lo)
    ncols = off

    # sc_pieces[S] = [(is_start, sub, plo, phi, slo), ...] emission order:
    # sub-major within a superchunk so same-stationary matmuls are adjacent
    sc_pieces = {c: [] for c in range(NSC)}
    tmp = {c: [] for c in range(NSC)}
    for c, plo, phi, off0, rlo, rw in pieces:
        tmp[c].append((plo, phi, off0, rlo, rw))
    slot_seen = set()
    for c in range(NSC):
        for sub in range(SUBS):
            for plo, phi, off0, rlo, rw in tmp[c]:
                slo = off0 + sub * rw + (plo - rlo)
                j = plo // 512
                is_start = j not in slot_seen
                slot_seen.add(j)
                sc_pieces[c].append((is_start, sub, plo, phi, slo))
    # spack DMA groups: one per superchunk
    grp_hi = []
    for jc in range(NSC):
        nxt = [run_off[i] for i, r in enumerate(runs) if r[0] > jc]
        grp_hi.append(min(nxt) if nxt else ncols)

    return dict(keys=keys, order=order, runs=runs, run_off=run_off,
                ncols=ncols, sc_pieces=sc_pieces, grp_hi=grp_hi,
                chunkmap=chunk)


# ---------------------------------------------------------------------------
# Host data packing (layout/index preprocessing only)
# ---------------------------------------------------------------------------

def _prep_inputs(tpl, features, w1, b1, w2, b2, w3, b3, edge_tf):
    bf = ml_dtypes.bfloat16
    keys, order, runs = tpl["keys"], tpl["order"], tpl["runs"]
    run_off, ncols = tpl["run_off"], tpl["ncols"]

    featT = np.repeat(np.ascontiguousarray(features.T), K, axis=0)
    featT = np.ascontiguousarray(
        featT.reshape(NCH, 128, B).transpose(1, 0, 2)).astype(bf)

    w1c = w1.reshape(T * K).reshape(NCH, 128).T.astype(np.float32)
    b1c = b1.reshape(T * K).reshape(NCH, 128).T.astype(np.float32)
    b2c = b2.reshape(T * K).reshape(NCH, 128).T.astype(np.float32)
    cols = np.concatenate([w1c, b1c, b2c], axis=1).copy()

    w2r = w2.reshape(NCH, 32, K, K)
    w2blk = np.zeros((NCH, 32, K, 32, K), np.float32)
    for i in range(32):
        w2blk[:, i, :, i, :] = w2r[:, i]
    w2blk = np.ascontiguousarray(
        w2blk.reshape(NCH, 128, 128).transpose(1, 0, 2)).astype(bf)

    # per-gene merged columns per distinct superchunk slot, per sub-chunk
    gcol = np.zeros((G, 3, SUBS, 128), np.float32)
    gidx = np.arange(G)
    for e in range(EPG):
        t = edge_tf[:, e]
        cc = t // 64
        s = np.argmax(keys == cc[:, None], axis=1)
        sub = (t % 64) // 32
        rows = 4 * (t % 32)
        for k in range(K):
            np.add.at(gcol, (gidx, s, sub, rows + k), w3[:, e, k])

    gcore = np.empty((NCORES, GSH), np.int64)      # position -> original gene
    for core in range(NCORES):
        gcore[core] = order[np.arange(GSH) * 8 + core]

    spack = np.zeros((NCORES, 128, ncols), np.float32)
    for ri, (c, kind, lo, hi, bk, lvl) in enumerate(runs):
        w = hi - lo
        o = run_off[ri]
        ps = np.arange(lo, hi)
        for core in range(NCORES):
            genes = gcore[core][ps]
            kk = keys[genes]
            member = kk[:, 0] == bk[0]
            for d in range(1, lvl):
                member &= kk[:, d] == bk[d]
            s = np.argmax(kk == c, axis=1)
            for sub in range(SUBS):
                vals = np.where(member[:, None], gcol[genes, s, sub, :], 0.0)
                spack[core, :, o + sub * w : o + (sub + 1) * w] = vals.T
    spack = spack.astype(bf)

    b3p = np.zeros((NCORES, 1, GSH), np.float32)
    for core in range(NCORES):
        b3p[core, 0, :] = b3[gcore[core]]
    b3p = b3p.astype(bf)

    in_maps = []
    for core in range(NCORES):
        in_maps.append({
            "featT": featT,
            "cols": cols,
            "W2blk": w2blk,
            "Spack": np.ascontiguousarray(spack[core]),
            "B3p": np.ascontiguousarray(b3p[core]),
        })
    return in_maps, gcore


# ---------------------------------------------------------------------------
# Graph
# ---------------------------------------------------------------------------

def _build_graph(tpl):
    from contextlib import ExitStack

    ncols = tpl["ncols"]
    sc_pieces = tpl["sc_pieces"]
    grp_hi = tpl["grp_hi"]

    nc = bass.Bass()
    featT_h = nc.declare_dram_parameter("featT", [128, NCH, B], BF16, isOutput=False)
    cols_h = nc.declare_dram_parameter("cols", [128, 3 * NCH], F32, isOutput=False)
    w2blk_h = nc.declare_dram_parameter("W2blk", [128, NCH, 128], BF16, isOutput=False)
    spack_h = nc.declare_dram_parameter("Spack", [128, ncols], BF16, isOutput=False)
    b3p_h = nc.declare_dram_parameter("B3p", [1, GSH], BF16, isOutput=False)
    out_h = nc.declare_dram_parameter("out", [B, GSH], BF16, isOutput=True)

    def slot_w(j):
        return min(GSH - 512 * j, 512)

    with ExitStack() as es:
        featT = es.enter_context(nc.sbuf_tensor("ft_sb", [128, NCH, B], BF16))
        colsb = es.enter_context(nc.sbuf_tensor("cols_sb", [128, 3 * NCH], F32))
        w2blk = es.enter_context(nc.sbuf_tensor("w2_sb", [128, NCH, 128], BF16))
        spk = es.enter_context(nc.sbuf_tensor("spk_sb", [128, ncols], BF16))
        b3sb = es.enter_context(nc.sbuf_tensor("b3_sb", [1, GSH], BF16))
        ones = es.enter_context(nc.sbuf_tensor("ones_sb", [1, 128], BF16))
        tbuf = es.enter_context(nc.sbuf_tensor("t_sb", [128, B], BF16))
        h1 = es.enter_context(nc.sbuf_tensor("h1_sb", [128, NCH, B], BF16))
        h2 = es.enter_context(nc.sbuf_tensor("h2_sb", [128, NCH, B], BF16))
        outsb = es.enter_context(nc.sbuf_tensor("out_sb", [128, NBT, 512 * NSLOT], BF16))
        touch = es.enter_context(nc.sbuf_tensor("touch_sb", [128, 4], BF16))
        pm = [es.enter_context(nc.psum_tensor(f"pm{j}", [128, 512], F32))
              for j in range(8)]

        w1a = colsb[:, 0:NCH]
        b1a = colsb[:, NCH : 2 * NCH]
        b2a = colsb[:, 2 * NCH : 3 * NCH]

        # DMA chain order on the single sync queue (baseline-proven sems):
        # cols, b3, w2blk, fq0, sp0, fq1, sp1, fq2, sp2, fq3, sp3..sp7
        FQ_INC = [16 * p for p in (4, 6, 8, 10)]          # featT quarter pos
        SP_INC = [16 * p for p in (5, 7, 9, 11, 12, 13, 14, 15)]

        with (
            nc.Block() as block,
            nc.semaphore("dsync") as dsync,    # single DMA chain
            nc.semaphore("peh") as sem_peh,    # PE w2-mm per chunk
            nc.semaphore("act") as sem_act,    # ACT h1/h2, 2 per chunk
            nc.semaphore("pem") as sem_pem,    # PE bank complete (b3-mm)
            nc.semaphore("ev") as sem_ev,      # DVE evictions (ordered)
            nc.semaphore("od") as sem_od,      # out DMA
        ):
            def ev_wait(engine, m, j):
                """Wait for the previous tenant of bank BANK(m,j) to evict."""
                prev = {(1, 3): (0, 0), (1, 4): (0, 1), (2, 0): (0, 2),
                        (2, 1): (0, 3), (2, 2): (0, 4), (2, 3): (1, 0),
                        (2, 4): (1, 1), (3, 0): (1, 2), (3, 1): (1, 3),
                        (3, 2): (1, 4), (3, 3): (2, 0), (3, 4): (2, 1)}.get((m, j))
                if prev is not None:
                    engine.wait_ge(sem_ev, EV_RANK[prev] + 1)

            @block.sync
            def _(sync: bass.BassEngine):
                sync.dma_start(out=colsb[:], in_=cols_h[:]).then_inc(dsync, 16)
                sync.dma_start(out=b3sb[:], in_=b3p_h[:]).then_inc(dsync, 16)
                sync.dma_start(out=w2blk[:], in_=w2blk_h[:]).then_inc(dsync, 16)
                sp_bounds = [0] + list(grp_hi)
                for q in range(4):
                    sync.dma_start(
                        out=featT[:, 4 * q : 4 * (q + 1), :],
                        in_=featT_h[:, 4 * q : 4 * (q + 1), :],
                    ).then_inc(dsync, 16)
                    lo, hi = sp_bounds[q], sp_bounds[q + 1]
                    sync.dma_start(
                        out=spk[:, lo : max(hi, lo + 1)],
                        in_=spack_h[:, lo : max(hi, lo + 1)],
                    ).then_inc(dsync, 16)
                for q in range(4, 8):
                    lo, hi = sp_bounds[q], sp_bounds[q + 1]
                    sync.dma_start(
                        out=spk[:, lo : max(hi, lo + 1)],
                        in_=spack_h[:, lo : max(hi, lo + 1)],
                    ).then_inc(dsync, 16)
                for e, (m, j) in enumerate(EV_LIST):
                    sync.wait_ge(sem_ev, e + 1)
                    w = slot_w(j)
                    sync.dma_start(
                        out=out_h[m * 128 : (m + 1) * 128, 512 * j : 512 * j + w],
                        in_=outsb[:, m, 512 * j : 512 * j + w],
                    ).then_inc(sem_od, 16)
                sync.wait_ge(sem_od, 16 * len(EV_LIST))

            @block.vector
            def _(vector: bass.BassEngine):
                vector.memset(ones[:], 1.0)
                for e, (m, j) in enumerate(EV_LIST):
                    w = slot_w(j)
                    vector.wait_ge(sem_pem, e + 1)
                    vector.tensor_scalar_add(
                        outsb[:, m, 512 * j : 512 * j + w],
                        pm[BANK(m, j)][:, :w], 0.0,
                    ).then_inc(sem_ev)

            @block.scalar
            def _(scalar: bass.BassEngine):
                # baseline-proven producer: ACT does both h1 (from SBUF) and
                # h2 (from the W2 psum), incrementing sem_act twice per chunk
                for c in range(NCH):
                    scalar.wait_ge(dsync, FQ_INC[c // 4])
                    scalar.activation(
                        h1[:, c, :], featT[:, c, :], AFT.Prelu,
                        bias=b1a[:, c : c + 1], scale=w1a[:, c : c + 1],
                        alpha=ALPHA,
                    ).then_inc(sem_act)
                    scalar.wait_ge(sem_peh, c + 1)
                    scalar.activation(
                        h2[:, c, :], pm[5 + c % 2][:, :], AFT.Prelu,
                        bias=b2a[:, c : c + 1], alpha=ALPHA,
                    ).then_inc(sem_act)

            @block.tensor
            def _(tensor: bass.BassEngine):
                def warm(k, n=512):
                    for _ in range(k):
                        tensor.matmul(
                            pm[7][:, :n], featT[:, 0, 0:128], featT[:, 0, :n],
                            start=True, stop=True, skip_group_check=True,
                        )

                def emit_runs(m, sc, slots):
                    for is_start, sub, plo, phi, slo in sc_pieces[sc]:
                        j = plo // 512
                        if j not in slots:
                            continue
                        w = phi - plo
                        tensor.matmul(
                            pm[BANK(m, j)][:, plo - 512 * j : phi - 512 * j],
                            h2[:, SUBS * sc + sub, m * 128 : (m + 1) * 128],
                            spk[:, slo : slo + w],
                            start=is_start, stop=False, skip_group_check=True,
                        )

                def b3mm(m, j):
                    w = slot_w(j)
                    tensor.matmul(
                        pm[BANK(m, j)][:, :w], ones[0:1, 0:128],
                        b3sb[0:1, 512 * j : 512 * j + w],
                        start=False, stop=True, skip_group_check=True,
                    ).then_inc(sem_pem)

                def w2mm(c):
                    if c == 0:
                        tensor.wait_ge(dsync, 48)       # w2blk
                    tensor.wait_ge(sem_act, 2 * c + 1)  # h1(c) written
                    tensor.matmul(
                        pm[5 + c % 2][:, :], w2blk[:, c, :], h1[:, c, :],
                        start=True, stop=True,
                    ).then_inc(sem_peh)

                warm(5)
                # build + btile0 (+ btile1's bank-7 slot j=2)
                for sc in range(NSC):
                    w2mm(2 * sc)
                    w2mm(2 * sc + 1)
                    tensor.wait_ge(sem_act, 2 * (2 * sc + 1) + 2)  # h2 ready
                    tensor.wait_ge(dsync, SP_INC[sc])   # spack group
                    emit_runs(0, sc, (0, 1, 2, 3, 4))
                    emit_runs(1, sc, (2,))
                tensor.wait_ge(dsync, 32)               # b3sb
                for j in range(5):
                    b3mm(0, j)
                b3mm(1, 2)
                # btile1 slots 0,1 (banks 5,6 -- free once ACT consumed ph)
                for sc in range(NSC):
                    emit_runs(1, sc, (0, 1))
                b3mm(1, 0)
                b3mm(1, 1)
                # btile1 slots 3,4 (banks 0,1 <- evictions of t0 j0,j1)
                ev_wait(tensor, 1, 3)
                ev_wait(tensor, 1, 4)
                for sc in range(NSC):
                    emit_runs(1, sc, (3, 4))
                b3mm(1, 3)
                b3mm(1, 4)
                # btile2
                for j in range(5):
                    ev_wait(tensor, 2, j)
                for sc in range(NSC):
                    emit_runs(2, sc, (0, 1, 2, 3, 4))
                for j in range(5):
                    b3mm(2, j)
                # btile3
                for j in range(5):
                    ev_wait(tensor, 3, j)
                for sc in range(NSC):
                    emit_runs(3, sc, (0, 1, 2, 3, 4))
                for j in range(5):
                    b3mm(3, j)

    return nc


def kernel(features, w1, b1, w2, b2, w3, b3, edge_tf):
    global LAST_RESULT
    features, w1, b1, w2, b2, w3, b3, edge_tf = (
        np.asarray(x) for x in (features, w1, b1, w2, b2, w3, b3, edge_tf)
    )
    key = hash(edge_tf.tobytes())
    if key not in _CACHE:
        tpl = _build_template(edge_tf)
        _CACHE.clear()
        _CACHE[key] = (tpl, _build_graph(tpl))
    tpl, graph = _CACHE[key]

    in_maps, gcore = _prep_inputs(
        tpl, features, w1, b1, w2, b2, w3, b3, edge_tf)
    trace = bool(int(os.environ.get("KERNEL_TRACE", "0")))
    if trace:
        _ensure_profile_hook()
    _enable_ldw_opt()
    res = run_bass_kernel_spmd(
        graph, in_maps, core_ids=list(range(NCORES)), trace=trace,
    )
    LAST_RESULT = res
    out = np.zeros((B, G), np.float32)
    for core in range(NCORES):
        dev = np.asarray(res.results[core]["out"]).astype(np.float32)
        out[:, gcore[core]] = dev
    return out
